# revision 25
# baseline (speedup 1.0000x reference)
"""Trainium2 Bass kernel for nn_Head_84043920048318 (sparse_attention).

Reference computation (per batch b):
    q = x @ Wq; k = x @ Wk; v = x @ Wv           [T, HS]
    wei = (q @ k.T) * C**-0.5                    [T, T]
    for s:  P = softmax(wei * adjacent[b, s], axis=-1);  out[b, s] = P @ v

Sharding: data-parallel over B across 8 NeuronCores (4 batches each);
projection weights replicated.

v8 design (from v7 @ ~113us):
  - adjacency is TRANSPOSED on the host (free) so the kernel computes
    wei^T = k @ q^T and the product P^T = wei^T * adj^T directly in
    partition=u layout: the 16 PE transposes per pair are gone, and the
    exp reads SBUF instead of PSUM.
  - x is supplied pre-transposed ([C, BPC, T]) so the projection chain
    needs no PE transposes either (make_identity deleted).
  - quad granularity: one DMA / one DVE multiply / one ACT exp per
    4 pairs (= half batch) instead of per pair. Cuts sem + init
    overhead on the two bottleneck engines (ACT exp floor ~57us,
    DVE mult+normalize ~60us).
  - output is fp16 in a p-major device layout ([BPC,128,2,4,TB,HS]):
    4KB contiguous runs per partition keep the DMA at full rate
    (<512B runs transfer at half rate), halving output traffic vs f32.
    Host unpacks to [B,S,T,HS] f32.
  - DMA split: adjacency (16.8MB/core) on the sync HWDGE ring; output
    stores on the (otherwise idle) GpSimd SWDGE ring.
  - normalize: av(PSUM f32) / den via a single DVE tensor_tensor divide
    (fused; no reciprocal op). Denominator comes from the ones column
    appended to v ([*, 129] matmul trick).

exp without max-subtraction is safe: |scale * wei * adj| <~ 8.
"""

import numpy as np
import ml_dtypes

B, S, T, C, HS = 32, 8, 512, 128, 128
NCORES = 8
BPC = B // NCORES
TB = T // 128
UB = T // 128
SCALE = float(C) ** -0.5

NQ = BPC * 2        # quads (half-batches) per core
SLICE_QUADS = 2     # first quads streamed per-slice to shorten pipeline fill
QLA = 4             # adjacency quad prefetch depth (= adjp bufs)
USE_DIVIDE = False  # DVE divide can't read both operands from PSUM (NCC_IBVF027)

_CACHED = None


def _build_module():
    import concourse.bacc as bacc
    import concourse.mybir as mybir
    from concourse import tile

    f32 = mybir.dt.float32
    f32r = mybir.dt.float32r
    bf16 = mybir.dt.bfloat16
    fp16 = mybir.dt.float16

    nc = bacc.Bacc("TRN2", target_bir_lowering=False, debug=False, num_devices=1)

    # xT: x pre-transposed on host -> [C, BPC, T]
    xT_d = nc.dram_tensor("xT", [C, BPC, T], bf16, kind="ExternalInput").ap()
    # adjacent: HOST-TRANSPOSED -> element [b, s, u, t]
    adj_d = nc.dram_tensor("adjacent", [BPC, S, T, T], bf16, kind="ExternalInput").ap()
    # w: Wq/Wk/Wv stacked -> [C, 3, HS]
    w_d = nc.dram_tensor("w", [C, 3, HS], bf16, kind="ExternalInput").ap()
    # out: p-major fp16; host unpacks. [b, p, si, sj, n, d] with t = n*128+p,
    # s = 4*si + sj.
    out_d = nc.dram_tensor(
        "out", [BPC, 128, 2, 4, TB, HS], fp16, kind="ExternalOutput"
    ).ap()

    with tile.TileContext(nc) as tc:
        with (
            tc.tile_pool(name="consts", bufs=1) as consts,
            tc.tile_pool(name="adjp", bufs=QLA) as adjp,
            tc.tile_pool(name="qkp", bufs=2) as qkp,
            tc.tile_pool(name="prodp", bufs=2) as prodp,
            tc.tile_pool(name="ptp", bufs=2) as ptp,
            tc.tile_pool(name="outp", bufs=2) as outp,
            tc.tile_pool(name="tiny", bufs=8) as tiny,
            tc.tile_pool(name="pgen", bufs=1, space="PSUM") as pgen,
            tc.tile_pool(name="pav", bufs=2, space="PSUM") as pav,
        ):
            xTt = consts.tile([C, BPC, T], bf16, tag="xT")
            nc.sync.dma_start(xTt[:, 0], xT_d[:, 0])
            wb = consts.tile([C, 3, HS], bf16, tag="wb")
            nc.sync.dma_start(wb[:], w_d)
            nc.sync.dma_start(xTt[:, 1:], xT_d[:, 1:])

            adj_tiles = {}

            def adj_load(q):
                b, si = q // 2, q % 2
                t = adjp.tile([128, 4, TB, T], bf16, tag="adj", name="adj")
                if q < SLICE_QUADS:
                    for sj in range(4):
                        nc.sync.dma_start(
                            t[:, sj],
                            adj_d[b, 4 * si + sj].rearrange(
                                "(n p) t -> p n t", p=128
                            ),
                        )
                else:
                    nc.sync.dma_start(
                        t[:],
                        adj_d[b, 4 * si : 4 * si + 4].rearrange(
                            "s (n p) t -> p s n t", p=128
                        ),
                    )
                adj_tiles[q] = t

            for q in range(min(QLA, NQ)):
                adj_load(q)

            wei_b, vp_b, proj_tmp = [None] * BPC, [None] * BPC, {}
            NPROJ = 4

            def proj_stage(bn, k):
                """Stage k (0..3) of batch bn's projections (q^T/k^T, wei^T, v).

                wei^T is written by the PE directly to PSUM as bf16 (pwei) —
                no evacuation copy; the DVE mult reads it from PSUM (one PSUM
                operand is legal, and 2x_1p only needs 2-byte dtypes). The
                qk/v transients borrow pav slots (2-bank each)."""
                if k == 0:
                    psAB = pav.tile([HS, 2, T], f32, tag="av", name="psAB")
                    nc.tensor.matmul(psAB[:, 0], wb[:, 0], xTt[:, bn])
                    nc.tensor.matmul(psAB[:, 1], wb[:, 1], xTt[:, bn])
                    proj_tmp["psAB"] = psAB
                elif k == 1:
                    qk = qkp.tile([HS, 2, T], f32r, tag="qk", name="qk")
                    nc.vector.tensor_copy(qk[:], proj_tmp.pop("psAB")[:])
                    proj_tmp["qk"] = qk
                    # wei^T[u, t] = sum_d k[u, d] q[t, d]: stationary k^T
                    # chunk, moving q^T. All 4 chunks into one 4-bank PSUM
                    # tile -> a single evacuation copy per batch.
                    w_ps = pgen.tile([128, TB, T], f32, tag="wps", name="w_ps")
                    for ub in range(TB):
                        nc.tensor.matmul(
                            w_ps[:, ub],
                            qk[:, 1, ub * 128 : (ub + 1) * 128],
                            qk[:, 0],
                        )
                    proj_tmp["w_ps"] = w_ps
                elif k == 2:
                    proj_tmp.pop("qk")
                    v_ps = pav.tile([128, UB, HS], f32, tag="av", name="v_ps")
                    for ub in range(UB):
                        nc.tensor.matmul(
                            v_ps[:, ub],
                            xTt[:, bn, ub * 128 : (ub + 1) * 128],
                            wb[:, 2],
                        )
                    proj_tmp["v_ps"] = v_ps
                elif k == 3:
                    # single ACT copy; emitted after exp(Q+1) in ACT program
                    # order (proj runs after the q==0 exps, and the sj2 exp
                    # was emitted in the previous iteration).
                    wei_b[bn] = consts.tile(
                        [128, TB, T], bf16, tag=f"wei{bn}", name=f"wei{bn}"
                    )
                    nc.scalar.copy(wei_b[bn][:], proj_tmp.pop("w_ps")[:])
                    vp = consts.tile(
                        [128, UB, HS + 1], bf16, tag=f"vp{bn}", name=f"vp{bn}"
                    )
                    nc.vector.tensor_copy(vp[:, :, 0:HS], proj_tmp.pop("v_ps")[:])
                    nc.vector.memset(vp[:, :, HS : HS + 1], 1.0)
                    vp_b[bn] = vp

            # batch 0 projected up front (nothing to hide under)
            for k in range(NPROJ):
                proj_stage(0, k)

            prods, pts, outb_g = {}, {}, {}

            def mult(q):
                # per-pair mults: measured cheaper on HW than one broadcast
                # quad op (4x1215ns vs 5300ns), and each gates only on its
                # own adjacency slice.
                b = q // 2
                prod = prodp.tile([128, 4, TB, T], bf16, tag="prod", name="prod")
                adj = adj_tiles.pop(q)
                for sj in range(4):
                    nc.vector.tensor_mul(prod[:, sj], adj[:, sj], wei_b[b][:])
                prods[q] = prod

            def expq(q, sj=None):
                # pair-level (sj given): first quad (warms ACT earlier in the
                # fill, interleaves with bn=1 wei copies) and last quad (AVs
                # start after the first 2us pair exp instead of the 7us quad
                # exp -> shorter drain).
                if sj is None or sj == 0:
                    pt = ptp.tile([128, 4, UB, T], bf16, tag="pt", name="pt")
                    pts[q] = pt
                pt = pts[q]
                if sj is None:
                    prod = prods.pop(q)
                    nc.scalar.activation(
                        pt[:], prod[:],
                        mybir.ActivationFunctionType.Exp, scale=SCALE,
                    )
                else:
                    prod = prods[q] if sj < 3 else prods.pop(q)
                    nc.scalar.activation(
                        pt[:, sj], prod[:, sj],
                        mybir.ActivationFunctionType.Exp, scale=SCALE,
                    )

            def finish(i):
                q, sj, b = i // 4, i % 4, i // 8
                si = q % 2
                if sj == 0:
                    outb_g[q] = outp.tile(
                        [128, 4, TB, HS], fp16, tag="outb", name="outb"
                    )
                pt = pts[q]
                av = pav.tile([128, TB, 256], f32, tag="av", name="av")
                for tb in range(TB):
                    for ub in range(UB):
                        nc.tensor.matmul(
                            av[:, tb, 0 : HS + 1],
                            pt[:, sj, ub, tb * 128 : (tb + 1) * 128],
                            vp_b[b][:, ub, :],
                            start=(ub == 0),
                            stop=(ub == UB - 1),
                        )
                if USE_DIVIDE:
                    nc.vector.tensor_tensor(
                        outb_g[q][:, sj],
                        av[:, :, 0:HS],
                        av[:, :, HS : HS + 1].broadcast_to([128, TB, HS]),
                        op=mybir.AluOpType.divide,
                    )
                else:
                    rcp = tiny.tile([128, TB], f32, tag="rcp", name="rcp")
                    nc.vector.reciprocal(rcp[:], av[:, :, HS : HS + 1])
                    nc.vector.tensor_mul(
                        outb_g[q][:, sj],
                        av[:, :, 0:HS],
                        rcp[:].unsqueeze(-1).broadcast_to([128, TB, HS]),
                    )
                if sj == 3:
                    pts.pop(q)

            mult(0)
            # batch bn's 4 proj stages: 1 per pair across its proj quad.
            # Quad choice balances pwei slot WAR (bufs=2: wei[bn] waits the
            # mults reading wei[bn-2]) against being ready for mult(2bn) at
            # quad 2bn-1 sj0.
            proj_quad = {1: 0, 2: 2, 3: 3}
            proj_sched = {}
            for bn in range(1, BPC):
                base = 4 * proj_quad[bn]
                for j in range(4):
                    proj_sched[base + j] = (bn, [j])
            for i in range(NQ * 4):
                q, sj = i // 4, i % 4
                if sj == 0:
                    if q + QLA < NQ:
                        adj_load(q + QLA)
                    # mult(q+1) must precede quad q's norms in DVE program
                    # order: it has no dependency on exp(q), while the norms
                    # do (via the AV matmuls) — emitting it first breaks the
                    # exp->AV->norm->mult->exp loop-carried chain.
                    if q + 1 < NQ:
                        mult(q + 1)
                if q == 0:
                    expq(0, sj)
                if i in proj_sched:
                    bn, ks = proj_sched[i]
                    for k in ks:
                        proj_stage(bn, k)
                if sj == 2 and q + 1 < NQ:
                    expq(q + 1, sj=None if q + 1 < NQ - 1 else 0)
                    if q + 1 == NQ - 1:
                        for sjj in range(1, 4):
                            expq(q + 1, sjj)
                if sj == 3 and q >= 1:
                    # emitted after exp(q+1) in ACT program order: by the time
                    # the DGE reaches the ACT queue head, the norms it waits
                    # on are long done (no head-of-line stall).
                    bp, sip = (q - 1) // 2, (q - 1) % 2
                    nc.scalar.dma_start(out_d[bp, :, sip], outb_g.pop(q - 1)[:])
                finish(i)
                if q == NQ - 1:
                    # last quad: pair-wise stores on the (now idle) sync ring
                    # right after each norm -> the final transfer is 1KB per
                    # partition instead of 4KB, cutting the drain tail.
                    ob = outb_g[q] if sj < 3 else outb_g.pop(q)
                    nc.sync.dma_start(out_d[BPC - 1, :, 1, sj], ob[:, sj])

    nc.compile()
    return nc


def _get_module():
    global _CACHED
    if _CACHED is None:
        _CACHED = _build_module()
    return _CACHED


def run_on_hw(in_maps, trace=False, trace_kwargs=None):
    """Run the compiled module on the 8 NeuronCores. Returns BassKernelResults."""
    from concourse.bass_utils import run_bass_kernel_spmd
    from concourse.bass_interp import get_hw_module

    nc = _get_module()
    old_m = nc.m
    nc.m = get_hw_module(nc.m)
    try:
        return run_bass_kernel_spmd(
            nc,
            in_maps,
            core_ids=list(range(NCORES)),
            trace=trace,
            **(trace_kwargs or {}),
        )
    finally:
        nc.m = old_m


def make_in_maps(x, adjacent, Wq, Wk, Wv):
    bf = ml_dtypes.bfloat16
    x = np.asarray(x, dtype=np.float32)
    adj = np.asarray(adjacent, dtype=np.float32).astype(bf)
    w = np.ascontiguousarray(
        np.stack(
            [np.asarray(Wq), np.asarray(Wk), np.asarray(Wv)], axis=1
        ).astype(bf)
    )
    maps = []
    for c in range(NCORES):
        xc = x[c * BPC : (c + 1) * BPC]                      # [BPC, T, C]
        xT = np.ascontiguousarray(xc.transpose(2, 0, 1).astype(bf))  # [C,BPC,T]
        adjT = np.ascontiguousarray(
            adj[c * BPC : (c + 1) * BPC].transpose(0, 1, 3, 2)
        )                                                    # [BPC,S,T,T] u-major
        maps.append({"xT": xT, "adjacent": adjT, "w": w})
    return maps


def _unpack_out(r):
    # [b, p, si, sj, n, d] -> [b, s=4*si+sj, t=n*128+p, d]
    return (
        r.transpose(0, 2, 3, 4, 1, 5)
        .reshape(BPC, S, T, HS)
        .astype(np.float32)
    )


def kernel(**inputs) -> np.ndarray:
    in_maps = make_in_maps(
        inputs["x"], inputs["adjacent"], inputs["Wq"], inputs["Wk"], inputs["Wv"]
    )
    res = run_on_hw(in_maps)
    return np.concatenate(
        [_unpack_out(res.results[c]["out"]) for c in range(NCORES)], axis=0
    )
